# revision 1
# baseline (speedup 1.0000x reference)
"""Trainium2 Bass kernel for GRU(I=8,H=6) + Linear(6->4) over [B=4096, T=512].

Pure data-parallel over 8 NeuronCores; B/8 = 512 rows per core.

Feature-major on-device layout: the per-core batch of 512 is packed as G=4
groups of 128 batch columns; weights are host-packed into block-diagonal
matrices so one PE pass covers all 4 groups. Every engine AP partition base
is 32-aligned (hardware requirement), so the PSUM gate tile uses 32-row
blocks: [xn @0:24 | hn @32:56 | r @64:88 | z @96:120] (pads zero-filled).

Per timestep t (128 batch columns per group):
  mm1 (PE):   ps[128,128] = Wx.T @ x_t[33,128]    x rows + ones row (biases)
  mm2 (PE):   ps         += Wh.T @ h[25,128]      h rows + ones row
  sig (ACT):  rz[64,128]  = sigmoid(ps[64:128])   r=rz[0:24], z=rz[32:56]
  u   (DVE):  u = rz[0:24] * ps[32:56]            r * hn
  mm_acc(PE): ps[0:24]   += I24.T @ u             xn + r*hn
  tanh(ACT):  n = tanh(ps[0:24])
  d (GPSIMD): d = h[0:24] - n
  e (GPSIMD): e = rz[32:56] * d                   z * (h - n)
  h'  (DVE):  h[0:24] = n + d*z                   new hidden state
  mm3 (PE):   po[16, (t%4)*128:...] = Wlin.T @ h  output projection
  every 4 steps: ACT copy po->SBUF, DMA -> DRAM out

Output leaves the device feature-major [T/4, 16, 512]; host reassembles to
[B, T, 4].
"""

import os
import sys

for _p in ("/opt/trn_rl_repo", "/root/.axon_site/_ro/trn_rl_repo"):
    if os.path.isdir(_p) and _p not in sys.path:
        sys.path.insert(0, _p)

import numpy as np

I, H, O = 8, 6, 4
B, T = 4096, 512
NCORES = 8
BS = B // NCORES        # 512 batch rows per core
G = 4                   # batch groups packed via block-diagonal weights
CB = BS // G            # 128 batch columns per group
GH = G * H              # 24
GI = G * I              # 32
GO = G * O              # 16

_CACHE = {}


def _build_module():
    import concourse.tile as tile
    from concourse import bacc, mybir
    from contextlib import ExitStack

    f32 = mybir.dt.float32
    Sig = mybir.ActivationFunctionType.Sigmoid
    Tanh = mybir.ActivationFunctionType.Tanh
    mult = mybir.AluOpType.mult
    add = mybir.AluOpType.add
    subtract = mybir.AluOpType.subtract

    nc = bacc.Bacc(
        "TRN2",
        target_bir_lowering=False,
        debug=False,
        enable_asserts=False,
        num_devices=NCORES,
    )

    xt_d = nc.dram_tensor("xt", [T, GI + 1, CB], f32, kind="ExternalInput").ap()
    wx_d = nc.dram_tensor("wx", [GI + 1, 128], f32, kind="ExternalInput").ap()
    wh_d = nc.dram_tensor("wh", [GH + 1, 128], f32, kind="ExternalInput").ap()
    wacc_d = nc.dram_tensor("wacc", [GH, GH], f32, kind="ExternalInput").ap()
    wlin_d = nc.dram_tensor("wlin", [GH + 1, GO], f32, kind="ExternalInput").ap()
    hinit_d = nc.dram_tensor("hinit", [GH + 1, CB], f32, kind="ExternalInput").ap()
    out_d = nc.dram_tensor("out", [T // 4, GO, 4 * CB], f32, kind="ExternalOutput").ap()

    with tile.TileContext(nc) as tc, ExitStack() as ctx:
        const = ctx.enter_context(tc.tile_pool(name="const", bufs=1))
        xpool = ctx.enter_context(tc.tile_pool(name="x", bufs=8))
        ps_pool = ctx.enter_context(tc.tile_pool(name="ps", bufs=2, space="PSUM"))
        po_pool = ctx.enter_context(tc.tile_pool(name="po", bufs=2, space="PSUM"))
        rz_pool = ctx.enter_context(tc.tile_pool(name="rz", bufs=3))
        n_pool = ctx.enter_context(tc.tile_pool(name="n", bufs=3))
        u_pool = ctx.enter_context(tc.tile_pool(name="u", bufs=3))
        d_pool = ctx.enter_context(tc.tile_pool(name="d", bufs=3))
        po_sb_pool = ctx.enter_context(tc.tile_pool(name="po_sb", bufs=2))
        hpool = ctx.enter_context(tc.tile_pool(name="h", bufs=1))

        wx_s = const.tile([GI + 1, 128], f32)
        nc.sync.dma_start(wx_s[:], wx_d)
        wh_s = const.tile([GH + 1, 128], f32)
        nc.sync.dma_start(wh_s[:], wh_d)
        wacc_s = const.tile([GH, GH], f32)
        nc.sync.dma_start(wacc_s[:], wacc_d)
        wlin_s = const.tile([GH + 1, GO], f32)
        nc.sync.dma_start(wlin_s[:], wlin_d)

        h_t = hpool.tile([GH + 1, CB], f32)
        nc.sync.dma_start(h_t[:], hinit_d)

        po = None
        for t in range(T):
            x_t = xpool.tile([GI + 1, CB], f32)
            nc.sync.dma_start(x_t[:], xt_d[t, :, :])

            ps = ps_pool.tile([128, CB], f32)
            nc.tensor.matmul(ps[:], wx_s[:], x_t[:], start=True, stop=False)
            nc.tensor.matmul(ps[:], wh_s[:], h_t[:], start=False, stop=False)

            # psum rows 64:128 hold [z @64:88 | r @96:120]; after the copy
            # z = rz[0:24] (base 0, matches d), r = rz[32:56] (base 32, matches hn)
            rz = rz_pool.tile([64, CB], f32)
            nc.scalar.activation(rz[:], ps[64:128, :], Sig)

            u = u_pool.tile([GH, CB], f32)
            nc.vector.tensor_tensor(out=u[:], in0=rz[32 : 32 + GH, :], in1=ps[32 : 32 + GH, :], op=mult)

            nc.tensor.matmul(ps[0:GH, :], wacc_s[:], u[:], start=False, stop=True)

            n_ = n_pool.tile([GH, CB], f32)
            nc.scalar.activation(n_[:], ps[0:GH, :], Tanh)

            d_ = d_pool.tile([GH, CB], f32)
            nc.gpsimd.tensor_tensor(out=d_[:], in0=h_t[0:GH, :], in1=n_[:], op=subtract)

            e_ = d_pool.tile([GH, CB], f32, tag="e")
            nc.gpsimd.tensor_tensor(out=e_[:], in0=rz[0:GH, :], in1=d_[:], op=mult)

            nc.vector.tensor_tensor(out=h_t[0:GH, :], in0=n_[:], in1=e_[:], op=add)

            tt = t % 4
            if tt == 0:
                po = po_pool.tile([GO, 4 * CB], f32)
            nc.tensor.matmul(
                po[:, tt * CB : (tt + 1) * CB], wlin_s[:], h_t[:], start=True, stop=True
            )
            if tt == 3:
                po_sb = po_sb_pool.tile([GO, 4 * CB], f32)
                nc.scalar.copy(po_sb[:], po[:])
                nc.sync.dma_start(out_d[t // 4, :, :], po_sb[:])

    nc.compile()
    return nc


def _pack_weights(W_ih, W_hh, b_ih, b_hh, W_lin, b_lin):
    # psum row blocks (32-aligned): xn @0, hn @32, r @64, z @96
    wx = np.zeros((GI + 1, 128), np.float32)
    wh = np.zeros((GH + 1, 128), np.float32)
    wlin = np.zeros((GH + 1, GO), np.float32)
    for g in range(G):
        sl_x = slice(g * I, (g + 1) * I)
        sl_h = slice(g * H, (g + 1) * H)
        # xn block: x weights + b_ih[n] on x ones-row
        wx[sl_x, 0 + g * H : 0 + (g + 1) * H] = W_ih[12:18].T
        wx[GI, 0 + g * H : 0 + (g + 1) * H] = b_ih[12:18]
        # hn block: h weights + b_hh[n] on h ones-row
        wh[sl_h, 32 + g * H : 32 + (g + 1) * H] = W_hh[12:18].T
        wh[GH, 32 + g * H : 32 + (g + 1) * H] = b_hh[12:18]
        # z block @64: both weights, biases on x ones-row
        wx[sl_x, 64 + g * H : 64 + (g + 1) * H] = W_ih[6:12].T
        wx[GI, 64 + g * H : 64 + (g + 1) * H] = b_ih[6:12] + b_hh[6:12]
        wh[sl_h, 64 + g * H : 64 + (g + 1) * H] = W_hh[6:12].T
        # r block @96
        wx[sl_x, 96 + g * H : 96 + (g + 1) * H] = W_ih[0:6].T
        wx[GI, 96 + g * H : 96 + (g + 1) * H] = b_ih[0:6] + b_hh[0:6]
        wh[sl_h, 96 + g * H : 96 + (g + 1) * H] = W_hh[0:6].T
        # linear projection
        wlin[sl_h, g * O : (g + 1) * O] = W_lin.T
        wlin[GH, g * O : (g + 1) * O] = b_lin
    wacc = np.eye(GH, dtype=np.float32)
    return wx, wh, wacc, wlin


def _run(inputs, trace=False):
    from concourse.bass_utils import run_bass_kernel_spmd

    x = np.ascontiguousarray(np.asarray(inputs["x"], dtype=np.float32))
    W_ih = np.asarray(inputs["W_ih"], np.float32)
    W_hh = np.asarray(inputs["W_hh"], np.float32)
    b_ih = np.asarray(inputs["b_ih"], np.float32)
    b_hh = np.asarray(inputs["b_hh"], np.float32)
    W_lin = np.asarray(inputs["W_lin"], np.float32)
    b_lin = np.asarray(inputs["b_lin"], np.float32)

    if "nc" not in _CACHE:
        _CACHE["nc"] = _build_module()
    nc = _CACHE["nc"]

    wx, wh, wacc, wlin = _pack_weights(W_ih, W_hh, b_ih, b_hh, W_lin, b_lin)
    hinit = np.zeros((GH + 1, CB), np.float32)
    hinit[GH, :] = 1.0

    in_maps = []
    for c in range(NCORES):
        xc = x[c * BS : (c + 1) * BS]                     # [512, 512, 8]
        xt = np.ones((T, GI + 1, CB), np.float32)
        xt[:, :GI, :] = xc.reshape(G, CB, T, I).transpose(2, 0, 3, 1).reshape(T, GI, CB)
        in_maps.append(
            {"xt": xt, "wx": wx, "wh": wh, "wacc": wacc, "wlin": wlin, "hinit": hinit}
        )

    res = run_bass_kernel_spmd(
        nc, in_maps, core_ids=list(range(NCORES)), trace=trace
    )

    outs = []
    for c in range(NCORES):
        a = res.results[c]["out"]                        # [T/4, 16, 512]
        a = a.reshape(T // 4, G, O, 4, CB)               # [t4, g, o, tt, b]
        a = a.transpose(1, 4, 0, 3, 2)                   # [g, b, t4, tt, o]
        outs.append(a.reshape(BS, T, O))
    full = np.concatenate(outs, axis=0)
    return full, res


def kernel(**inputs) -> np.ndarray:
    out, _ = _run(inputs, trace=False)
    return out


def kernel_profiled(inputs):
    """Returns (output, BassKernelResults-with-trace)."""
    return _run(inputs, trace=True)



# revision 12
# speedup vs baseline: 1.5315x; 1.5315x over previous
"""Trainium2 Bass kernel for GRU(I=8,H=6) + Linear(6->4) over [B=4096, T=512].

Data-parallel over 8 NeuronCores; 512 batch rows per core, packed as 4
groups of 128 batch columns (fp16 on-device, fp32 PSUM accumulate).

h_t lives in a persistent SBUF "mega" tile rows 0:24, one 128-col block
per timestep (block t = h_{t-1}); the full history doubles as the moving
operand for a deferred output-projection phase. PSUM blocks per step:
V@0:24 | HN@32:56 | R@64:88 | Z@96:120 (V = xn, then += u).

Loop-carried chain per step (no weight reload on the chain; partition
bases chosen so every tensor-tensor op has co-based inputs and every
matmul sits at tile position (0,0) like the stock kernels):
  MM_h (PE)   psum += Uh' @ h_{t-1}                 gates' h-side
  sigr (ACT)  r = sigmoid(psum[R] + bias) -> r@32:56
  u    (DVE)  u = (psum[HN@32] + b_hh_n) * r@32         fused stt
  MM_B (PE)   psum[V@0] += I @ u
  tanh (ACT)  n = tanh(psum[V@0] + b_ih_n) -> n@0:24
  e    (DVE)  e = (z@0 - 1) * n@0                       fused stt
  h    (DVE)  h_t = f - e -> mega block t+1
Off-path: MM_x (x-side matmul, start=True, double-buffered psum),
sigz (ACT) z = sigmoid(psum[Z]) -> z@0:24, f = z*h_{t-1} on GPSIMD,
x DMA in 16-step chunks. Output projection streams the mega history
through the PE at the end (4 step-groups stacked per PSUM bank via
column tile_position), one ACT copy per 16 steps, fp16 DMA out.
"""

import os
import sys

for _p in ("/opt/trn_rl_repo", "/root/.axon_site/_ro/trn_rl_repo"):
    if os.path.isdir(_p) and _p not in sys.path:
        sys.path.insert(0, _p)

import numpy as np

I, H, O = 8, 6, 4
B, T = 4096, 512
NCORES = 8
BS = B // NCORES        # 512 batch rows per core
G = 4                   # batch groups packed on partitions
CB = BS // G            # 128 batch columns per group
GH = G * H              # 24
GI = G * I              # 32
GO = G * O              # 16

_CACHE = {}


def _build_module():
    import concourse.tile as tile
    from concourse import bacc, mybir
    from contextlib import ExitStack

    f16 = mybir.dt.float16
    f32 = mybir.dt.float32
    Sig = mybir.ActivationFunctionType.Sigmoid
    Tanh = mybir.ActivationFunctionType.Tanh
    Ident = mybir.ActivationFunctionType.Identity
    add = mybir.AluOpType.add
    mult = mybir.AluOpType.mult
    subtract = mybir.AluOpType.subtract

    nc = bacc.Bacc(
        "TRN2",
        target_bir_lowering=False,
        debug=False,
        enable_asserts=False,
        num_devices=NCORES,
    )

    NBLK = T + 1            # mega col-blocks: block t holds h_{t-1}
    XCH = 16                # timesteps per x DMA chunk

    xf_d = nc.dram_tensor("xf", [GI, T * CB], f16, kind="ExternalInput").ap()
    wh_d = nc.dram_tensor("whs", [GH, 128], f16, kind="ExternalInput").ap()
    wx_d = nc.dram_tensor("wxs", [GI, 128], f16, kind="ExternalInput").ap()
    wi_d = nc.dram_tensor("wis", [GH, GH], f16, kind="ExternalInput").ap()
    wp_d = nc.dram_tensor("wps", [GH, 32], f16, kind="ExternalInput").ap()
    bias_d = nc.dram_tensor("bias", [128, 1], f32, kind="ExternalInput").ap()
    pbias_d = nc.dram_tensor("pbias", [128, 1], f32, kind="ExternalInput").ap()
    out_d = nc.dram_tensor("out", [T // 4, GO, 4 * CB], f16, kind="ExternalOutput").ap()

    with tile.TileContext(nc) as tc, ExitStack() as ctx:
        const = ctx.enter_context(tc.tile_pool(name="const", bufs=1))
        mega_pool = ctx.enter_context(tc.tile_pool(name="mega", bufs=1))
        xpool = ctx.enter_context(tc.tile_pool(name="x", bufs=3))
        ps_pool = ctx.enter_context(tc.tile_pool(name="ps", bufs=3, space="PSUM"))
        r_pool = ctx.enter_context(tc.tile_pool(name="r", bufs=3))
        z_pool = ctx.enter_context(tc.tile_pool(name="z", bufs=3))
        n_pool = ctx.enter_context(tc.tile_pool(name="n", bufs=3))
        u_pool = ctx.enter_context(tc.tile_pool(name="u", bufs=3))
        f_pool = ctx.enter_context(tc.tile_pool(name="f", bufs=2))
        pp_pool = ctx.enter_context(tc.tile_pool(name="pp", bufs=2, space="PSUM"))
        ob_pool = ctx.enter_context(tc.tile_pool(name="ob", bufs=2))

        wh_s = const.tile([GH, 128], f16)
        nc.sync.dma_start(wh_s[:], wh_d)
        wx_s = const.tile([GI, 128], f16)
        nc.sync.dma_start(wx_s[:], wx_d)
        wi_s = const.tile([GH, GH], f16)
        nc.sync.dma_start(wi_s[:], wi_d)
        wp_s = const.tile([GH, 32], f16)
        nc.sync.dma_start(wp_s[:], wp_d)
        bias_s = const.tile([128, 1], f32)
        nc.sync.dma_start(bias_s[:], bias_d)
        pbias_s = const.tile([128, 1], f32)
        nc.sync.dma_start(pbias_s[:], pbias_d)

        mega = mega_pool.tile([GH, NBLK * CB], f16)
        nc.vector.memset(mega[:, 0:CB], 0.0)       # h_{-1} = 0

        def blk(t):
            return slice(t * CB, (t + 1) * CB)

        xtiles = {}

        def fetch_chunk(c):
            if c * XCH >= T or c in xtiles:
                return
            xt = xpool.tile([GI, XCH * CB], f16, name="xt", tag="xt")
            nc.sync.dma_start(
                xt[:], xf_d[:, c * XCH * CB : (c + 1) * XCH * CB]
            )
            xtiles[c] = xt

        fetch_chunk(0)
        fetch_chunk(1)

        def mm_x(t, ps):
            c, s = t // XCH, t % XCH
            xt = xtiles[c]
            nc.tensor.matmul(
                ps[0:128, :],
                wx_s[:],
                xt[:, s * CB : (s + 1) * CB],
                start=True,
                stop=False,
            )

        def new_ps():
            return ps_pool.tile([128, CB], f32, name="ps", tag="ps")

        ps_cur = new_ps()
        mm_x(0, ps_cur)

        for t in range(T):
            if t % XCH == 0:
                fetch_chunk(t // XCH + 2)

            # h-side gates: psum += Uh' @ h_{t-1}
            nc.tensor.matmul(
                ps_cur[0:128, :], wh_s[:], mega[:, blk(t)],
                start=False, stop=False,
            )

            # x-side for step t+1 (off critical path, separate psum bank)
            if t + 1 < T:
                ps_next = new_ps()
                mm_x(t + 1, ps_next)
            else:
                ps_next = None

            r_t = r_pool.tile([56, CB], f16)
            nc.scalar.activation(
                r_t[32:56, :], ps_cur[64:88, :], Sig, bias=bias_s[64:88]
            )

            u_t = u_pool.tile([GH, CB], f16)
            nc.vector.scalar_tensor_tensor(
                out=u_t[:], in0=ps_cur[32:56, :], scalar=bias_s[32:56],
                in1=r_t[32:56, :], op0=add, op1=mult,
            )

            nc.tensor.matmul(
                ps_cur[0:24, :], wi_s[:], u_t[:],
                start=False, stop=True,
            )

            # z-gate sigmoid: needed only after tanh -> off critical path
            z_t = z_pool.tile([GH, CB], f16)
            nc.scalar.activation(
                z_t[:], ps_cur[96:120, :], Sig, bias=bias_s[96:120]
            )

            # f = z * h_{t-1} on GPSIMD (off critical path)
            f_t = f_pool.tile([GH, CB], f16)
            nc.gpsimd.tensor_tensor(
                out=f_t[:], in0=z_t[:], in1=mega[:, blk(t)], op=mult,
            )

            n_t = n_pool.tile([GH, CB], f16)
            nc.scalar.activation(
                n_t[:], ps_cur[0:24, :], Tanh, bias=bias_s[0:24]
            )

            # e = (z - 1) * n
            e_t = n_pool.tile([GH, CB], f16, name="e_t", tag="e")
            nc.vector.scalar_tensor_tensor(
                out=e_t[:], in0=z_t[:], scalar=1.0,
                in1=n_t[:], op0=subtract, op1=mult,
            )

            # h_t = f - e -> mega block t+1
            nc.vector.tensor_tensor(
                out=mega[:, blk(t + 1)], in0=f_t[:], in1=e_t[:], op=subtract,
            )

            ps_cur = ps_next

        # ---- output projection: y_t = Wlin @ h_t (+ b via ACT bias) ----
        for c in range(T // XCH):
            pp = pp_pool.tile([128, 4 * CB], f32, name="pp", tag="pp")
            for j in range(4):
                t0 = c * XCH + 4 * j           # steps t0..t0+3
                nc.tensor.matmul(
                    pp[32 * j : 32 * j + 32, :],
                    wp_s[:],
                    mega[:, (t0 + 1) * CB : (t0 + 5) * CB],
                    start=True, stop=True,
                    tile_position=(0, 32 * j),
                )
            ob = ob_pool.tile([128, 4 * CB], f16, name="ob", tag="ob")
            nc.scalar.activation(ob[:], pp[:], Ident, bias=pbias_s[:])
            for j in range(4):
                nc.sync.dma_start(
                    out_d[c * 4 + j, :, :], ob[32 * j : 32 * j + GO, :]
                )

    nc.compile()
    return nc


def _pack_weights(W_ih, W_hh, b_ih, b_hh, W_lin, b_lin):
    # PSUM blocks: V@0:24 | HN@32:56 | R@64:88 | Z@96:120
    wh = np.zeros((GH, 128), np.float32)      # h rows (mega 0:24)
    wx = np.zeros((GI, 128), np.float32)      # x rows (0:32)
    wp = np.zeros((GH, 32), np.float32)
    bias = np.zeros((128, 1), np.float32)
    pbias = np.zeros((128, 1), np.float32)
    Ur, Uz, Un = W_hh[0:6], W_hh[6:12], W_hh[12:18]
    Wr, Wz, Wn = W_ih[0:6], W_ih[6:12], W_ih[12:18]
    for g in range(G):
        hsl = slice(g * H, (g + 1) * H)
        wh[hsl, 32 + g * H : 38 + g * H] = Un.T
        wh[hsl, 64 + g * H : 70 + g * H] = Ur.T
        wh[hsl, 96 + g * H : 102 + g * H] = Uz.T
        xsl = slice(g * I, (g + 1) * I)
        wx[xsl, 0 + g * H : 6 + g * H] = Wn.T     # V = xn
        wx[xsl, 64 + g * H : 70 + g * H] = Wr.T
        wx[xsl, 96 + g * H : 102 + g * H] = Wz.T
        wp[hsl, g * O : (g + 1) * O] = W_lin.T
        bias[0 + g * H : 6 + g * H, 0] = b_ih[12:18]            # tanh V bias
        bias[32 + g * H : 38 + g * H, 0] = b_hh[12:18]          # u stt scalar
        bias[64 + g * H : 70 + g * H, 0] = b_ih[0:6] + b_hh[0:6]     # r
        bias[96 + g * H : 102 + g * H, 0] = b_ih[6:12] + b_hh[6:12]  # z
        for j in range(4):
            pbias[32 * j + g * O : 32 * j + (g + 1) * O, 0] = b_lin
    wi = np.eye(GH, dtype=np.float32)
    return (
        wh.astype(np.float16),
        wx.astype(np.float16),
        wi.astype(np.float16),
        wp.astype(np.float16),
        bias,
        pbias,
    )


def _run(inputs, trace=False):
    from concourse.bass_utils import run_bass_kernel_spmd

    x = np.asarray(inputs["x"], dtype=np.float32)
    W_ih = np.asarray(inputs["W_ih"], np.float32)
    W_hh = np.asarray(inputs["W_hh"], np.float32)
    b_ih = np.asarray(inputs["b_ih"], np.float32)
    b_hh = np.asarray(inputs["b_hh"], np.float32)
    W_lin = np.asarray(inputs["W_lin"], np.float32)
    b_lin = np.asarray(inputs["b_lin"], np.float32)

    if "nc" not in _CACHE:
        _CACHE["nc"] = _build_module()
    nc = _CACHE["nc"]

    wh, wx, wi, wp, bias, pbias = _pack_weights(W_ih, W_hh, b_ih, b_hh, W_lin, b_lin)

    in_maps = []
    for c in range(NCORES):
        xc = x[c * BS : (c + 1) * BS]                    # [512, 512, 8]
        xf = (
            xc.reshape(G, CB, T, I)
            .transpose(0, 3, 2, 1)                       # [g, i, t, b]
            .reshape(GI, T * CB)
            .astype(np.float16)
        )
        in_maps.append(
            {"xf": xf, "whs": wh, "wxs": wx, "wis": wi, "wps": wp,
             "bias": bias, "pbias": pbias}
        )

    res = run_bass_kernel_spmd(
        nc, in_maps, core_ids=list(range(NCORES)), trace=trace
    )

    outs = []
    for c in range(NCORES):
        a = res.results[c]["out"].astype(np.float32)     # [T/4, 16, 512]
        a = a.reshape(T // 4, G, O, 4, CB)               # [t4, g, o, tt, b]
        a = a.transpose(1, 4, 0, 3, 2)                   # [g, b, t4, tt, o]
        outs.append(a.reshape(BS, T, O))
    full = np.concatenate(outs, axis=0)
    return full, res


def kernel(**inputs) -> np.ndarray:
    out, _ = _run(inputs, trace=False)
    return out


def kernel_profiled(inputs):
    """Returns (output, BassKernelResults-with-trace)."""
    return _run(inputs, trace=True)


# revision 13
# speedup vs baseline: 1.8894x; 1.2337x over previous
"""Trainium2 Bass kernel for GRU(I=8,H=6) + Linear(6->4) over [B=4096, T=512].

Data-parallel over 8 NeuronCores; 512 batch rows per core, packed as 4
groups of 128 batch columns (fp16 on-device, fp32 PSUM accumulate).

The hidden state is carried as two fp16 pieces f_t = z_t*h_{t-1} and
e_t = (z_t-1)*n_t (h_t = f_t - e_t). The gates' h-side contribution for
step t+1 is accumulated by TWO small matmuls MM_f (+U @ f, off the
critical path) and MM_e (-U @ e, the only post-tanh chain op) into the
next step's PSUM bank, so h itself never sits on the loop-carried chain.
PSUM blocks per step: V@0:24 | HN@32:56 | Z@64:88 | R@96:120
(V = xn, then += u). All matmuls sit at PE tile row 0.

Loop-carried chain per step:
  MM_e (PE)   psum_{t} -= Ue' @ e_{t-1}       last h-side contribution
  sig  (ACT)  rz = sigmoid(psum[64:120]+bias) -> z@0:24, r@32:56
  u    (DVE)  u = (psum[HN@32] + b_hh_n) * r@32        fused stt
  MM_B (PE)   psum[V@0] += I @ u
  tanh (ACT)  n = tanh(psum[V@0] + b_ih_n) -> n@0:24
  e    (DVE)  e = z1 * n                                (z1 = z-1, off-path)
Off-path: MM_x (start=True), MM_f, f = z*hroll on GPSIMD, hroll = f-e on
DVE, mega h-history = f-e on GPSIMD (feeds the deferred projection),
x DMA in 16-step chunks. Output projection streams the h history through
the PE at the end (4 step-groups per PSUM bank via column tile_position),
one ACT copy per 16 steps, fp16 DMA out.
"""

import os
import sys

for _p in ("/opt/trn_rl_repo", "/root/.axon_site/_ro/trn_rl_repo"):
    if os.path.isdir(_p) and _p not in sys.path:
        sys.path.insert(0, _p)

import numpy as np

I, H, O = 8, 6, 4
B, T = 4096, 512
NCORES = 8
BS = B // NCORES        # 512 batch rows per core
G = 4                   # batch groups packed on partitions
CB = BS // G            # 128 batch columns per group
GH = G * H              # 24
GI = G * I              # 32
GO = G * O              # 16

_CACHE = {}


def _build_module():
    import concourse.tile as tile
    from concourse import bacc, mybir
    from contextlib import ExitStack

    f16 = mybir.dt.float16
    f32 = mybir.dt.float32
    Sig = mybir.ActivationFunctionType.Sigmoid
    Tanh = mybir.ActivationFunctionType.Tanh
    Ident = mybir.ActivationFunctionType.Identity
    add = mybir.AluOpType.add
    mult = mybir.AluOpType.mult
    subtract = mybir.AluOpType.subtract

    nc = bacc.Bacc(
        "TRN2",
        target_bir_lowering=False,
        debug=False,
        enable_asserts=False,
        num_devices=NCORES,
    )

    NBLK = T + 1            # mega col-blocks: block t+1 holds h_t
    XCH = 16                # timesteps per x DMA chunk

    xf_d = nc.dram_tensor("xf", [GI, T * CB], f16, kind="ExternalInput").ap()
    whf_d = nc.dram_tensor("whf", [GH, 128], f16, kind="ExternalInput").ap()
    whe_d = nc.dram_tensor("whe", [GH, 128], f16, kind="ExternalInput").ap()
    wx_d = nc.dram_tensor("wxs", [GI, 128], f16, kind="ExternalInput").ap()
    wi_d = nc.dram_tensor("wis", [GH, GH], f16, kind="ExternalInput").ap()
    wp_d = nc.dram_tensor("wps", [GH, 32], f16, kind="ExternalInput").ap()
    bias_d = nc.dram_tensor("bias", [128, 1], f32, kind="ExternalInput").ap()
    pbias_d = nc.dram_tensor("pbias", [128, 1], f32, kind="ExternalInput").ap()
    out_d = nc.dram_tensor("out", [T // 4, GO, 4 * CB], f16, kind="ExternalOutput").ap()

    with tile.TileContext(nc) as tc, ExitStack() as ctx:
        const = ctx.enter_context(tc.tile_pool(name="const", bufs=1))
        mega_pool = ctx.enter_context(tc.tile_pool(name="mega", bufs=1))
        xpool = ctx.enter_context(tc.tile_pool(name="x", bufs=3))
        ps_pool = ctx.enter_context(tc.tile_pool(name="ps", bufs=3, space="PSUM"))
        rz_pool = ctx.enter_context(tc.tile_pool(name="rz", bufs=3))
        z1_pool = ctx.enter_context(tc.tile_pool(name="z1", bufs=3))
        n_pool = ctx.enter_context(tc.tile_pool(name="n", bufs=3))
        u_pool = ctx.enter_context(tc.tile_pool(name="u", bufs=3))
        e_pool = ctx.enter_context(tc.tile_pool(name="e", bufs=3))
        f_pool = ctx.enter_context(tc.tile_pool(name="f", bufs=3))
        hr_pool = ctx.enter_context(tc.tile_pool(name="hr", bufs=3))
        pp_pool = ctx.enter_context(tc.tile_pool(name="pp", bufs=2, space="PSUM"))
        ob_pool = ctx.enter_context(tc.tile_pool(name="ob", bufs=2))

        whf_s = const.tile([GH, 128], f16)
        nc.sync.dma_start(whf_s[:], whf_d)
        whe_s = const.tile([GH, 128], f16)
        nc.sync.dma_start(whe_s[:], whe_d)
        wx_s = const.tile([GI, 128], f16)
        nc.sync.dma_start(wx_s[:], wx_d)
        wi_s = const.tile([GH, GH], f16)
        nc.sync.dma_start(wi_s[:], wi_d)
        wp_s = const.tile([GH, 32], f16)
        nc.sync.dma_start(wp_s[:], wp_d)
        bias_s = const.tile([128, 1], f32)
        nc.sync.dma_start(bias_s[:], bias_d)
        pbias_s = const.tile([128, 1], f32)
        nc.sync.dma_start(pbias_s[:], pbias_d)

        mega = mega_pool.tile([GH, NBLK * CB], f16)

        def blk(t):
            return slice(t * CB, (t + 1) * CB)

        xtiles = {}

        def fetch_chunk(c):
            if c * XCH >= T or c in xtiles:
                return
            xt = xpool.tile([GI, XCH * CB], f16, name="xt", tag="xt")
            nc.sync.dma_start(
                xt[:], xf_d[:, c * XCH * CB : (c + 1) * XCH * CB]
            )
            xtiles[c] = xt

        fetch_chunk(0)
        fetch_chunk(1)

        def mm_x(t, ps):
            c, s = t // XCH, t % XCH
            xt = xtiles[c]
            nc.tensor.matmul(
                ps[0:128, :],
                wx_s[:],
                xt[:, s * CB : (s + 1) * CB],
                start=True,
                stop=False,
            )

        def new_ps():
            return ps_pool.tile([128, CB], f32, name="ps", tag="ps")

        hroll_prev = hr_pool.tile([GH, CB], f16, name="hroll", tag="hr")
        nc.vector.memset(hroll_prev[:], 0.0)    # h_{-1} = 0

        ps_cur = new_ps()
        mm_x(0, ps_cur)

        e_prev = None
        for t in range(T):
            if t % XCH == 0:
                fetch_chunk(t // XCH + 2)

            # last h-side contribution for step t (on the chain)
            if e_prev is not None:
                nc.tensor.matmul(
                    ps_cur[0:128, :], whe_s[:], e_prev[:],
                    start=False, stop=False,
                )

            # combined r+z sigmoid: psum Z@64->z@0:24, R@96->r@32:56
            rz = rz_pool.tile([56, CB], f16)
            nc.scalar.activation(
                rz[0:56, :], ps_cur[64:120, :], Sig, bias=bias_s[64:120]
            )

            u_t = u_pool.tile([GH, CB], f16)
            nc.vector.scalar_tensor_tensor(
                out=u_t[:], in0=ps_cur[32:56, :], scalar=bias_s[32:56],
                in1=rz[32:56, :], op0=add, op1=mult,
            )

            nc.tensor.matmul(
                ps_cur[0:24, :], wi_s[:], u_t[:],
                start=False, stop=True,
            )

            # x-side for step t+1 (off critical path, next psum bank)
            if t + 1 < T:
                ps_next = new_ps()
                mm_x(t + 1, ps_next)
            else:
                ps_next = None

            # z1 = z - 1 (DVE, fills the tanh wait window)
            z1_t = z1_pool.tile([GH, CB], f16)
            nc.vector.tensor_scalar(
                out=z1_t[:], in0=rz[0:24, :], scalar1=1.0, scalar2=None,
                op0=subtract,
            )

            # f = z * h_{t-1} on GPSIMD (off critical path)
            f_t = f_pool.tile([GH, CB], f16)
            nc.gpsimd.tensor_tensor(
                out=f_t[:], in0=rz[0:24, :], in1=hroll_prev[:], op=mult,
            )

            n_t = n_pool.tile([GH, CB], f16)
            nc.scalar.activation(
                n_t[:], ps_cur[0:24, :], Tanh, bias=bias_s[0:24]
            )

            # e = (z - 1) * n  (chain) -> feeds MM_e into next psum bank
            e_t = e_pool.tile([GH, CB], f16)
            nc.vector.tensor_tensor(
                out=e_t[:], in0=z1_t[:], in1=n_t[:], op=mult,
            )

            if ps_next is not None:
                # f-side h contribution for step t+1 (off critical path)
                nc.tensor.matmul(
                    ps_next[0:128, :], whf_s[:], f_t[:],
                    start=False, stop=False,
                )

            # rolling h for the next f (off critical path)
            hroll_t = hr_pool.tile([GH, CB], f16, name="hroll", tag="hr")
            nc.vector.tensor_tensor(
                out=hroll_t[:], in0=f_t[:], in1=e_t[:], op=subtract,
            )
            hroll_prev = hroll_t

            # h history for the projection (off critical path)
            nc.gpsimd.tensor_tensor(
                out=mega[:, blk(t + 1)], in0=f_t[:], in1=e_t[:], op=subtract,
            )

            e_prev = e_t
            ps_cur = ps_next

        # ---- output projection: y_t = Wlin @ h_t (+ b via ACT bias) ----
        for c in range(T // XCH):
            pp = pp_pool.tile([128, 4 * CB], f32, name="pp", tag="pp")
            for j in range(4):
                t0 = c * XCH + 4 * j           # steps t0..t0+3
                nc.tensor.matmul(
                    pp[32 * j : 32 * j + 32, :],
                    wp_s[:],
                    mega[:, (t0 + 1) * CB : (t0 + 5) * CB],
                    start=True, stop=True,
                    tile_position=(0, 32 * j),
                )
            ob = ob_pool.tile([128, 4 * CB], f16, name="ob", tag="ob")
            nc.scalar.activation(ob[:], pp[:], Ident, bias=pbias_s[:])
            for j in range(4):
                nc.sync.dma_start(
                    out_d[c * 4 + j, :, :], ob[32 * j : 32 * j + GO, :]
                )

    nc.compile()
    return nc


def _pack_weights(W_ih, W_hh, b_ih, b_hh, W_lin, b_lin):
    # PSUM blocks: V@0:24 | HN@32:56 | Z@64:88 | R@96:120
    whf = np.zeros((GH, 128), np.float32)     # f rows: +U
    wx = np.zeros((GI, 128), np.float32)
    wp = np.zeros((GH, 32), np.float32)
    bias = np.zeros((128, 1), np.float32)
    pbias = np.zeros((128, 1), np.float32)
    Ur, Uz, Un = W_hh[0:6], W_hh[6:12], W_hh[12:18]
    Wr, Wz, Wn = W_ih[0:6], W_ih[6:12], W_ih[12:18]
    for g in range(G):
        hsl = slice(g * H, (g + 1) * H)
        whf[hsl, 32 + g * H : 38 + g * H] = Un.T
        whf[hsl, 64 + g * H : 70 + g * H] = Uz.T
        whf[hsl, 96 + g * H : 102 + g * H] = Ur.T
        xsl = slice(g * I, (g + 1) * I)
        wx[xsl, 0 + g * H : 6 + g * H] = Wn.T     # V = xn
        wx[xsl, 64 + g * H : 70 + g * H] = Wz.T
        wx[xsl, 96 + g * H : 102 + g * H] = Wr.T
        wp[hsl, g * O : (g + 1) * O] = W_lin.T
        bias[0 + g * H : 6 + g * H, 0] = b_ih[12:18]            # tanh V bias
        bias[32 + g * H : 38 + g * H, 0] = b_hh[12:18]          # u stt scalar
        bias[64 + g * H : 70 + g * H, 0] = b_ih[6:12] + b_hh[6:12]   # z
        bias[96 + g * H : 102 + g * H, 0] = b_ih[0:6] + b_hh[0:6]    # r
        for j in range(4):
            pbias[32 * j + g * O : 32 * j + (g + 1) * O, 0] = b_lin
    whe = -whf
    wi = np.eye(GH, dtype=np.float32)
    return (
        whf.astype(np.float16),
        whe.astype(np.float16),
        wx.astype(np.float16),
        wi.astype(np.float16),
        wp.astype(np.float16),
        bias,
        pbias,
    )


def _run(inputs, trace=False):
    from concourse.bass_utils import run_bass_kernel_spmd

    x = np.asarray(inputs["x"], dtype=np.float32)
    W_ih = np.asarray(inputs["W_ih"], np.float32)
    W_hh = np.asarray(inputs["W_hh"], np.float32)
    b_ih = np.asarray(inputs["b_ih"], np.float32)
    b_hh = np.asarray(inputs["b_hh"], np.float32)
    W_lin = np.asarray(inputs["W_lin"], np.float32)
    b_lin = np.asarray(inputs["b_lin"], np.float32)

    if "nc" not in _CACHE:
        _CACHE["nc"] = _build_module()
    nc = _CACHE["nc"]

    whf, whe, wx, wi, wp, bias, pbias = _pack_weights(
        W_ih, W_hh, b_ih, b_hh, W_lin, b_lin
    )

    in_maps = []
    for c in range(NCORES):
        xc = x[c * BS : (c + 1) * BS]                    # [512, 512, 8]
        xf = (
            xc.reshape(G, CB, T, I)
            .transpose(0, 3, 2, 1)                       # [g, i, t, b]
            .reshape(GI, T * CB)
            .astype(np.float16)
        )
        in_maps.append(
            {"xf": xf, "whf": whf, "whe": whe, "wxs": wx, "wis": wi,
             "wps": wp, "bias": bias, "pbias": pbias}
        )

    res = run_bass_kernel_spmd(
        nc, in_maps, core_ids=list(range(NCORES)), trace=trace
    )

    outs = []
    for c in range(NCORES):
        a = res.results[c]["out"].astype(np.float32)     # [T/4, 16, 512]
        a = a.reshape(T // 4, G, O, 4, CB)               # [t4, g, o, tt, b]
        a = a.transpose(1, 4, 0, 3, 2)                   # [g, b, t4, tt, o]
        outs.append(a.reshape(BS, T, O))
    full = np.concatenate(outs, axis=0)
    return full, res


def kernel(**inputs) -> np.ndarray:
    out, _ = _run(inputs, trace=False)
    return out


def kernel_profiled(inputs):
    """Returns (output, BassKernelResults-with-trace)."""
    return _run(inputs, trace=True)


# revision 14
# speedup vs baseline: 2.1371x; 1.1311x over previous
"""Trainium2 Bass kernel for GRU(I=8,H=6) + Linear(6->4) over [B=4096, T=512].

Data-parallel over 8 NeuronCores; 512 batch rows per core, packed as 4
groups of 128 batch columns (fp16 on-device, fp32 PSUM accumulate).

The hidden state is carried as two fp16 pieces f_t = z_t*h_{t-1} and
e_t = (z_t-1)*n_t (h_t = f_t - e_t). The gates' h-side contribution for
step t+1 is accumulated by TWO small matmuls MM_f (+U @ f, off the
critical path) and MM_e (-U @ e, the only post-tanh chain op) into the
next step's PSUM bank, so h itself never sits on the loop-carried chain.
PSUM blocks per step: V@0:24 | HN@32:56 | Z@64:88 | R@96:120
(V = xn, then += u). All matmuls sit at PE tile row 0.

Loop-carried chain per step:
  MM_e (PE)   psum_{t} -= Ue' @ e_{t-1}       last h-side contribution
  sig  (ACT)  rz = sigmoid(psum[64:120]+bias) -> z@0:24, r@32:56
  u    (DVE)  u = (psum[HN@32] + b_hh_n) * r@32        fused stt
  MM_B (PE)   psum[V@0] += I @ u
  tanh (ACT)  n = tanh(psum[V@0] + b_ih_n) -> n@0:24
  e    (DVE)  e = z1 * n                                (z1 = z-1, off-path)
Off-path: MM_x (start=True), MM_f, f = z*hroll on GPSIMD, hroll = f-e on
DVE, mega h-history = f-e on GPSIMD (feeds the deferred projection),
x DMA in 16-step chunks. Output projection streams the h history through
the PE at the end (4 step-groups per PSUM bank via column tile_position),
one ACT copy per 16 steps, fp16 DMA out.
"""

import os
import sys

for _p in ("/opt/trn_rl_repo", "/root/.axon_site/_ro/trn_rl_repo"):
    if os.path.isdir(_p) and _p not in sys.path:
        sys.path.insert(0, _p)

import numpy as np

I, H, O = 8, 6, 4
B, T = 4096, 512
NCORES = 8
BS = B // NCORES        # 512 batch rows per core
G = 4                   # batch groups packed on partitions
CB = BS // G            # 128 batch columns per group
GH = G * H              # 24
GI = G * I              # 32
GO = G * O              # 16

_CACHE = {}


def _build_module():
    import concourse.tile as tile
    from concourse import bacc, mybir
    from concourse.instruction_name_ordered_set import InstructionNameOrderedSet
    from contextlib import ExitStack

    f16 = mybir.dt.float16
    f32 = mybir.dt.float32
    Sig = mybir.ActivationFunctionType.Sigmoid
    Tanh = mybir.ActivationFunctionType.Tanh
    Ident = mybir.ActivationFunctionType.Identity
    add = mybir.AluOpType.add
    mult = mybir.AluOpType.mult
    subtract = mybir.AluOpType.subtract

    nc = bacc.Bacc(
        "TRN2",
        target_bir_lowering=False,
        debug=False,
        enable_asserts=False,
        num_devices=NCORES,
    )

    NBLK = T + 1            # mega col-blocks: block t+1 holds h_t
    XCH = 16                # timesteps per x DMA chunk

    xf_d = nc.dram_tensor("xf", [GI, T * CB], f16, kind="ExternalInput").ap()
    whf_d = nc.dram_tensor("whf", [GH, 128], f16, kind="ExternalInput").ap()
    whe_d = nc.dram_tensor("whe", [GH, 128], f16, kind="ExternalInput").ap()
    wx_d = nc.dram_tensor("wxs", [GI, 128], f16, kind="ExternalInput").ap()
    wi_d = nc.dram_tensor("wis", [GH, GH], f16, kind="ExternalInput").ap()
    wp_d = nc.dram_tensor("wps", [GH, 32], f16, kind="ExternalInput").ap()
    bias_d = nc.dram_tensor("bias", [128, 1], f32, kind="ExternalInput").ap()
    pbias_d = nc.dram_tensor("pbias", [128, 1], f32, kind="ExternalInput").ap()
    out_d = nc.dram_tensor("out", [T // 4, GO, 4 * CB], f16, kind="ExternalOutput").ap()

    with tile.TileContext(nc) as tc, ExitStack() as ctx:
        const = ctx.enter_context(tc.tile_pool(name="const", bufs=1))
        mega_pool = ctx.enter_context(tc.tile_pool(name="mega", bufs=1))
        xpool = ctx.enter_context(tc.tile_pool(name="x", bufs=3))
        ps_pool = ctx.enter_context(tc.tile_pool(name="ps", bufs=3, space="PSUM"))
        rz_pool = ctx.enter_context(tc.tile_pool(name="rz", bufs=3))
        z1_pool = ctx.enter_context(tc.tile_pool(name="z1", bufs=3))
        n_pool = ctx.enter_context(tc.tile_pool(name="n", bufs=3))
        u_pool = ctx.enter_context(tc.tile_pool(name="u", bufs=3))
        e_pool = ctx.enter_context(tc.tile_pool(name="e", bufs=3))
        f_pool = ctx.enter_context(tc.tile_pool(name="f", bufs=3))
        hr_pool = ctx.enter_context(tc.tile_pool(name="hr", bufs=3))
        pp_pool = ctx.enter_context(tc.tile_pool(name="pp", bufs=2, space="PSUM"))
        ob_pool = ctx.enter_context(tc.tile_pool(name="ob", bufs=2))

        whf_s = const.tile([GH, 128], f16)
        nc.sync.dma_start(whf_s[:], whf_d)
        whe_s = const.tile([GH, 128], f16)
        nc.sync.dma_start(whe_s[:], whe_d)
        wx_s = const.tile([GI, 128], f16)
        nc.sync.dma_start(wx_s[:], wx_d)
        wi_s = const.tile([GH, GH], f16)
        nc.sync.dma_start(wi_s[:], wi_d)
        wp_s = const.tile([GH, 32], f16)
        nc.sync.dma_start(wp_s[:], wp_d)
        bias_s = const.tile([128, 1], f32)
        nc.sync.dma_start(bias_s[:], bias_d)
        pbias_s = const.tile([128, 1], f32)
        nc.sync.dma_start(pbias_s[:], pbias_d)

        mega = mega_pool.tile([GH, NBLK * CB], f16)

        def blk(t):
            return slice(t * CB, (t + 1) * CB)

        xtiles = {}

        def fetch_chunk(c):
            if c * XCH >= T or c in xtiles:
                return
            xt = xpool.tile([GI, XCH * CB], f16, name="xt", tag="xt")
            nc.sync.dma_start(
                xt[:], xf_d[:, c * XCH * CB : (c + 1) * XCH * CB]
            )
            xtiles[c] = xt

        fetch_chunk(0)
        fetch_chunk(1)

        def mm_x(t, ps):
            c, s = t // XCH, t % XCH
            xt = xtiles[c]
            nc.tensor.matmul(
                ps[0:128, :],
                wx_s[:],
                xt[:, s * CB : (s + 1) * CB],
                start=True,
                stop=False,
            )

        def new_ps():
            return ps_pool.tile([128, CB], f32, name="ps", tag="ps")

        hroll_prev = hr_pool.tile([GH, CB], f16, name="hroll", tag="hr")
        nc.vector.memset(hroll_prev[:], 0.0)    # h_{-1} = 0

        ps_cur = new_ps()
        mm_x(0, ps_cur)

        e_prev = None
        for t in range(T):
            if t % XCH == 0:
                fetch_chunk(t // XCH + 2)

            # last h-side contribution for step t (on the chain)
            if e_prev is not None:
                nc.tensor.matmul(
                    ps_cur[0:128, :], whe_s[:], e_prev[:],
                    start=False, stop=False,
                )

            # combined r+z sigmoid: psum Z@64->z@0:24, R@96->r@32:56
            rz = rz_pool.tile([56, CB], f16)
            nc.scalar.activation(
                rz[0:56, :], ps_cur[64:120, :], Sig, bias=bias_s[64:120]
            )

            u_t = u_pool.tile([GH, CB], f16)
            nc.vector.scalar_tensor_tensor(
                out=u_t[:], in0=ps_cur[32:56, :], scalar=bias_s[32:56],
                in1=rz[32:56, :], op0=add, op1=mult,
            )

            mmb = nc.tensor.matmul(
                ps_cur[0:24, :], wi_s[:], u_t[:],
                start=False, stop=True,
            )

            # x-side for step t+1 (off critical path, next psum bank)
            if t + 1 < T:
                ps_next = new_ps()
                mm_x(t + 1, ps_next)
            else:
                ps_next = None

            # z1 = z - 1 (DVE, fills the tanh wait window)
            z1_t = z1_pool.tile([GH, CB], f16)
            nc.vector.tensor_scalar(
                out=z1_t[:], in0=rz[0:24, :], scalar1=1.0, scalar2=None,
                op0=subtract,
            )

            # f = z * h_{t-1} on GPSIMD (off critical path)
            f_t = f_pool.tile([GH, CB], f16)
            nc.gpsimd.tensor_tensor(
                out=f_t[:], in0=rz[0:24, :], in1=hroll_prev[:], op=mult,
            )

            n_t = n_pool.tile([GH, CB], f16)
            nc.scalar.activation(
                n_t[:], ps_cur[0:24, :], Tanh, bias=bias_s[0:24]
            )

            # e = (z - 1) * n  (chain) -> feeds MM_e into next psum bank
            e_t = e_pool.tile([GH, CB], f16)
            nc.vector.tensor_tensor(
                out=e_t[:], in0=z1_t[:], in1=n_t[:], op=mult,
            )

            if ps_next is not None:
                # f-side h contribution for step t+1 (off critical path).
                # Order it after MM_B on the PE so it cannot block the chain
                # while waiting for the slow GPSIMD f-op.
                mmf = nc.tensor.matmul(
                    ps_next[0:128, :], whf_s[:], f_t[:],
                    start=False, stop=False,
                )
                deps = InstructionNameOrderedSet()
                deps.add(mmb.ins.name)
                mmf.ins.add_nosync_dependencies_from(deps)

            # rolling h for the next f (off critical path)
            hroll_t = hr_pool.tile([GH, CB], f16, name="hroll", tag="hr")
            nc.vector.tensor_tensor(
                out=hroll_t[:], in0=f_t[:], in1=e_t[:], op=subtract,
            )
            hroll_prev = hroll_t

            # h history for the projection (off critical path)
            nc.gpsimd.tensor_tensor(
                out=mega[:, blk(t + 1)], in0=f_t[:], in1=e_t[:], op=subtract,
            )

            e_prev = e_t
            ps_cur = ps_next

        # ---- output projection: y_t = Wlin @ h_t (+ b via ACT bias) ----
        for c in range(T // XCH):
            pp = pp_pool.tile([128, 4 * CB], f32, name="pp", tag="pp")
            for j in range(4):
                t0 = c * XCH + 4 * j           # steps t0..t0+3
                nc.tensor.matmul(
                    pp[32 * j : 32 * j + 32, :],
                    wp_s[:],
                    mega[:, (t0 + 1) * CB : (t0 + 5) * CB],
                    start=True, stop=True,
                    tile_position=(0, 32 * j),
                )
            ob = ob_pool.tile([128, 4 * CB], f16, name="ob", tag="ob")
            nc.scalar.activation(ob[:], pp[:], Ident, bias=pbias_s[:])
            for j in range(4):
                nc.sync.dma_start(
                    out_d[c * 4 + j, :, :], ob[32 * j : 32 * j + GO, :]
                )

    nc.compile()
    return nc


def _pack_weights(W_ih, W_hh, b_ih, b_hh, W_lin, b_lin):
    # PSUM blocks: V@0:24 | HN@32:56 | Z@64:88 | R@96:120
    whf = np.zeros((GH, 128), np.float32)     # f rows: +U
    wx = np.zeros((GI, 128), np.float32)
    wp = np.zeros((GH, 32), np.float32)
    bias = np.zeros((128, 1), np.float32)
    pbias = np.zeros((128, 1), np.float32)
    Ur, Uz, Un = W_hh[0:6], W_hh[6:12], W_hh[12:18]
    Wr, Wz, Wn = W_ih[0:6], W_ih[6:12], W_ih[12:18]
    for g in range(G):
        hsl = slice(g * H, (g + 1) * H)
        whf[hsl, 32 + g * H : 38 + g * H] = Un.T
        whf[hsl, 64 + g * H : 70 + g * H] = Uz.T
        whf[hsl, 96 + g * H : 102 + g * H] = Ur.T
        xsl = slice(g * I, (g + 1) * I)
        wx[xsl, 0 + g * H : 6 + g * H] = Wn.T     # V = xn
        wx[xsl, 64 + g * H : 70 + g * H] = Wz.T
        wx[xsl, 96 + g * H : 102 + g * H] = Wr.T
        wp[hsl, g * O : (g + 1) * O] = W_lin.T
        bias[0 + g * H : 6 + g * H, 0] = b_ih[12:18]            # tanh V bias
        bias[32 + g * H : 38 + g * H, 0] = b_hh[12:18]          # u stt scalar
        bias[64 + g * H : 70 + g * H, 0] = b_ih[6:12] + b_hh[6:12]   # z
        bias[96 + g * H : 102 + g * H, 0] = b_ih[0:6] + b_hh[0:6]    # r
        for j in range(4):
            pbias[32 * j + g * O : 32 * j + (g + 1) * O, 0] = b_lin
    whe = -whf
    wi = np.eye(GH, dtype=np.float32)
    return (
        whf.astype(np.float16),
        whe.astype(np.float16),
        wx.astype(np.float16),
        wi.astype(np.float16),
        wp.astype(np.float16),
        bias,
        pbias,
    )


def _run(inputs, trace=False):
    from concourse.bass_utils import run_bass_kernel_spmd

    x = np.asarray(inputs["x"], dtype=np.float32)
    W_ih = np.asarray(inputs["W_ih"], np.float32)
    W_hh = np.asarray(inputs["W_hh"], np.float32)
    b_ih = np.asarray(inputs["b_ih"], np.float32)
    b_hh = np.asarray(inputs["b_hh"], np.float32)
    W_lin = np.asarray(inputs["W_lin"], np.float32)
    b_lin = np.asarray(inputs["b_lin"], np.float32)

    if "nc" not in _CACHE:
        _CACHE["nc"] = _build_module()
    nc = _CACHE["nc"]

    whf, whe, wx, wi, wp, bias, pbias = _pack_weights(
        W_ih, W_hh, b_ih, b_hh, W_lin, b_lin
    )

    in_maps = []
    for c in range(NCORES):
        xc = x[c * BS : (c + 1) * BS]                    # [512, 512, 8]
        xf = (
            xc.reshape(G, CB, T, I)
            .transpose(0, 3, 2, 1)                       # [g, i, t, b]
            .reshape(GI, T * CB)
            .astype(np.float16)
        )
        in_maps.append(
            {"xf": xf, "whf": whf, "whe": whe, "wxs": wx, "wis": wi,
             "wps": wp, "bias": bias, "pbias": pbias}
        )

    res = run_bass_kernel_spmd(
        nc, in_maps, core_ids=list(range(NCORES)), trace=trace
    )

    outs = []
    for c in range(NCORES):
        a = res.results[c]["out"].astype(np.float32)     # [T/4, 16, 512]
        a = a.reshape(T // 4, G, O, 4, CB)               # [t4, g, o, tt, b]
        a = a.transpose(1, 4, 0, 3, 2)                   # [g, b, t4, tt, o]
        outs.append(a.reshape(BS, T, O))
    full = np.concatenate(outs, axis=0)
    return full, res


def kernel(**inputs) -> np.ndarray:
    out, _ = _run(inputs, trace=False)
    return out


def kernel_profiled(inputs):
    """Returns (output, BassKernelResults-with-trace)."""
    return _run(inputs, trace=True)
